# revision 4
# baseline (speedup 1.0000x reference)
import os
import sys
import subprocess
import tempfile
import functools
import numpy as np

# Hardcoded model dims (from problem spec)
S, B, W = 128, 32, 16
VOCAB, CHAR_VOCAB = 50000, 100
E, CE, FN, K = 300, 32, 4, 3
H, LAYERS, HEADS, NTAGS = 512, 2, 8, 20
CONV_OUT = CE * FN
NEG = -1e9
NCORES = 8
BSH = B // NCORES  # 4 sequences per core
DEV_TIMEOUT_S = 420


def _model_fns():
    import jax
    import jax.numpy as jnp

    def _lstm_dir(x, Wih, Whh, bih, bhh, reverse):
        Bn = x.shape[1]
        h0 = jnp.zeros((Bn, H), x.dtype)
        c0 = jnp.zeros((Bn, H), x.dtype)

        def step(carry, xt):
            h, c = carry
            g = xt @ Wih.T + bih + h @ Whh.T + bhh
            i, f, gg, o = jnp.split(g, 4, axis=-1)
            c = jax.nn.sigmoid(f) * c + jax.nn.sigmoid(i) * jnp.tanh(gg)
            h = jax.nn.sigmoid(o) * jnp.tanh(c)
            return (h, c), h

        _, hs = jax.lax.scan(step, (h0, c0), x, reverse=reverse)
        return hs

    def _mha(x, params, kpm):
        Wqkv, bqkv, Wo, bo = params
        Sn, Bn, Em = x.shape
        Dh = Em // HEADS
        qkv = x @ Wqkv.T + bqkv
        q, k, v = jnp.split(qkv, 3, axis=-1)

        def heads(t):
            return t.reshape(Sn, Bn, HEADS, Dh).transpose(1, 2, 0, 3)

        q, k, v = heads(q), heads(k), heads(v)
        scores = jnp.einsum('bhqd,bhkd->bhqk', q, k) / jnp.sqrt(jnp.float32(Dh))
        scores = jnp.where(kpm[:, None, None, :], NEG, scores)
        attn = jax.nn.softmax(scores, axis=-1)
        out = jnp.einsum('bhqk,bhkd->bhqd', attn, v).transpose(2, 0, 1, 3).reshape(Sn, Bn, Em)
        out = out @ Wo.T + bo
        return out, attn.mean(axis=1)

    def _core(words, chars, word_emb, char_emb_tab, conv_w, conv_b,
              lstm_params, attn_params, fc_w, fc_b):
        we = word_emb[words]
        ce = char_emb_tab[chars]
        Bn, Sn, Wn, CEn = ce.shape
        x = ce.reshape(Bn * Sn, Wn, CEn).transpose(0, 2, 1)
        conv = jax.lax.conv_general_dilated(
            x, conv_w, (1,), 'VALID',
            dimension_numbers=('NCH', 'OIH', 'NCH'), feature_group_count=CEn)
        conv = conv + conv_b[None, :, None]
        char_feat = conv.max(axis=2).reshape(Bn, Sn, -1).transpose(1, 0, 2)
        feat = jnp.concatenate([we, char_feat], axis=-1)
        out = feat
        for layer in lstm_params:
            fwd = _lstm_dir(out, *layer[:4], reverse=False)
            bwd = _lstm_dir(out, *layer[4:], reverse=True)
            out = jnp.concatenate([fwd, bwd], axis=-1)
        kpm = (words == 0).T
        attn_out, attn_w = _mha(out, attn_params, kpm)
        emissions = attn_out @ fc_w.T + fc_b
        return emissions, attn_w

    def _crf_llh_sum(emissions, tags, mask, start, end, trans):
        Sn, Bn, T = emissions.shape
        ar = jnp.arange(Bn)
        mf = mask.astype(emissions.dtype)
        first = start[tags[0]] + emissions[0, ar, tags[0]]

        def num_step(sc, inp):
            em, tp, tc, m = inp
            return sc + (trans[tp, tc] + em[ar, tc]) * m, None

        num, _ = jax.lax.scan(num_step, first,
                              (emissions[1:], tags[:-1], tags[1:], mf[1:]))
        seq_ends = mask.sum(0).astype(jnp.int32) - 1
        num = num + end[tags[seq_ends, ar]]

        def den_step(sc, inp):
            em, m = inp
            nxt = jax.nn.logsumexp(sc[:, :, None] + trans[None], axis=1) + em
            return jnp.where(m[:, None], nxt, sc), None

        den0 = start[None, :] + emissions[0]
        den, _ = jax.lax.scan(den_step, den0, (emissions[1:], mask[1:]))
        den = jax.nn.logsumexp(den + end[None, :], axis=1)
        return jnp.sum(num - den)

    def _viterbi(emissions, mask, start, end, trans):
        Sn, Bn, T = emissions.shape
        idT = jnp.arange(T, dtype=jnp.int32)

        def step(sc, inp):
            em, m = inp
            cand = sc[:, :, None] + trans[None]
            best_prev = jnp.argmax(cand, axis=1).astype(jnp.int32)
            nxt = jnp.max(cand, axis=1) + em
            sc_new = jnp.where(m[:, None], nxt, sc)
            hist = jnp.where(m[:, None], best_prev, idT[None, :])
            return sc_new, hist

        sc0 = start[None, :] + emissions[0]
        sc, hists = jax.lax.scan(step, sc0, (emissions[1:], mask[1:]))
        last = jnp.argmax(sc + end[None, :], axis=1).astype(jnp.int32)

        def back(tag, hist):
            prev = jnp.take_along_axis(hist, tag[:, None], axis=1)[:, 0]
            return prev, tag

        first_tag, rest = jax.lax.scan(back, last, hists, reverse=True)
        return jnp.concatenate([first_tag[None], rest], axis=0)

    def shard_forward(words, chars, tags, word_emb, char_emb_tab, conv_w, conv_b,
                      lstm_flat, attn_params, fc_w, fc_b,
                      crf_start, crf_end, crf_trans):
        lstm_params = (tuple(lstm_flat[0:8]), tuple(lstm_flat[8:16]))
        emissions, attn_w = _core(words, chars, word_emb, char_emb_tab,
                                  conv_w, conv_b, lstm_params,
                                  (attn_params[0], attn_params[1],
                                   attn_params[2], attn_params[3]),
                                  fc_w, fc_b)
        mask = words != 0
        crf_out = _viterbi(emissions, mask, crf_start, crf_end, crf_trans)
        crf_loss = -_crf_llh_sum(emissions, tags, mask,
                                 crf_start, crf_end, crf_trans)
        return crf_out, crf_loss, attn_w

    return shard_forward


def _shard_np(words, chars, tags):
    words_sh = np.stack([words[:, i * BSH:(i + 1) * BSH] for i in range(NCORES)])
    tags_sh = np.stack([tags[:, i * BSH:(i + 1) * BSH] for i in range(NCORES)])
    chars_sh = np.stack([chars[i * BSH:(i + 1) * BSH] for i in range(NCORES)])
    return words_sh, chars_sh, tags_sh


def _run_device(arrs):
    """Runs inside the child process: pmap across the 8 NeuronCores."""
    import jax
    shard_forward = _model_fns()
    words_sh, chars_sh, tags_sh = _shard_np(arrs['words'], arrs['chars'], arrs['tags'])
    lstm_flat = tuple(arrs[f'lstm{i}'] for i in range(16))
    attn_params = tuple(arrs[f'attn{i}'] for i in range(4))
    devs = jax.devices()[:NCORES]
    pf = jax.pmap(shard_forward, in_axes=(0, 0, 0) + (None,) * 11, devices=devs)
    crf_sh, loss_sh, attn_sh = pf(
        words_sh, chars_sh, tags_sh,
        arrs['word_emb'], arrs['char_emb_tab'], arrs['conv_w'], arrs['conv_b'],
        lstm_flat, attn_params, arrs['fc_w'], arrs['fc_b'],
        arrs['crf_start'], arrs['crf_end'], arrs['crf_trans'])
    return np.asarray(crf_sh), np.asarray(loss_sh), np.asarray(attn_sh)


def _sig(x):
    return 1.0 / (1.0 + np.exp(-x))


def _run_cpu(arrs):
    """Pure-numpy forward pass (no jax): guaranteed-correct fallback."""
    words, chars, tags = arrs['words'], arrs['chars'], arrs['tags']
    we = arrs['word_emb'][words]                                  # [S,B,E]
    ce = arrs['char_emb_tab'][chars]                              # [B,S,W,CE]
    cw = arrs['conv_w'].reshape(CE, FN, K)                        # grouped conv
    x = ce.transpose(0, 1, 3, 2).reshape(B * S, CE, W)
    conv = np.zeros((B * S, CE, FN, W - K + 1), np.float32)
    for k in range(K):
        conv += cw[None, :, :, k, None] * x[:, :, None, k:k + W - K + 1]
    conv = conv.reshape(B * S, CONV_OUT, W - K + 1) + arrs['conv_b'][None, :, None]
    char_feat = conv.max(axis=2).reshape(B, S, CONV_OUT).transpose(1, 0, 2)
    out = np.concatenate([we, char_feat], axis=-1).astype(np.float32)
    for l in range(LAYERS):
        p = [arrs[f'lstm{l * 8 + j}'] for j in range(8)]
        hs = []
        for d in range(2):
            Wih, Whh, bih, bhh = p[d * 4:d * 4 + 4]
            xproj = out @ Wih.T + bih + bhh                       # [S,B,4H]
            h = np.zeros((B, H), np.float32)
            c = np.zeros((B, H), np.float32)
            seq = range(S) if d == 0 else range(S - 1, -1, -1)
            hd = np.zeros((S, B, H), np.float32)
            for t in seq:
                g = xproj[t] + h @ Whh.T
                i_, f_, g_, o_ = g[:, :H], g[:, H:2 * H], g[:, 2 * H:3 * H], g[:, 3 * H:]
                c = _sig(f_) * c + _sig(i_) * np.tanh(g_)
                h = _sig(o_) * np.tanh(c)
                hd[t] = h
            hs.append(hd)
        out = np.concatenate(hs, axis=-1)
    # MHA
    Wqkv, bqkv, Wo, bo = [arrs[f'attn{i}'] for i in range(4)]
    Em = 2 * H
    Dh = Em // HEADS
    qkv = out @ Wqkv.T + bqkv
    q, k, v = qkv[..., :Em], qkv[..., Em:2 * Em], qkv[..., 2 * Em:]

    def heads(t):
        return t.reshape(S, B, HEADS, Dh).transpose(1, 2, 0, 3)

    q, k, v = heads(q), heads(k), heads(v)
    kpm = (words == 0).T                                          # [B,S]
    scores = np.einsum('bhqd,bhkd->bhqk', q, k) / np.sqrt(np.float32(Dh))
    scores = np.where(kpm[:, None, None, :], np.float32(NEG), scores)
    scores -= scores.max(axis=-1, keepdims=True)
    attn = np.exp(scores)
    attn /= attn.sum(axis=-1, keepdims=True)
    attn_w = attn.mean(axis=1)                                    # [B,S,S]
    mo = np.einsum('bhqk,bhkd->bhqd', attn, v).transpose(2, 0, 1, 3).reshape(S, B, Em)
    mo = mo @ Wo.T + bo
    emissions = mo @ arrs['fc_w'].T + arrs['fc_b']                # [S,B,T]
    mask = words != 0
    start, end, trans = arrs['crf_start'], arrs['crf_end'], arrs['crf_trans']
    ar = np.arange(B)
    # numerator
    num = start[tags[0]] + emissions[0, ar, tags[0]]
    mf = mask.astype(np.float32)
    for t in range(1, S):
        num = num + (trans[tags[t - 1], tags[t]] + emissions[t, ar, tags[t]]) * mf[t]
    seq_ends = mask.sum(0).astype(np.int32) - 1
    num = num + end[tags[seq_ends, ar]]
    # denominator
    den = start[None, :] + emissions[0]
    for t in range(1, S):
        m = den.max(axis=1, keepdims=True)                        # [B,1]
        nxt = np.log(np.exp(den[:, :, None] + trans[None] - m[:, :, None])
                     .sum(axis=1)) + m + emissions[t]
        den = np.where(mask[t][:, None], nxt, den)
    mx = den + end[None, :]
    m = mx.max(axis=1)
    den_f = np.log(np.exp(mx - m[:, None]).sum(axis=1)) + m
    loss = -np.sum(num - den_f)
    # viterbi
    sc = start[None, :] + emissions[0]
    hists = np.zeros((S - 1, B, NTAGS), np.int32)
    idT = np.arange(NTAGS, dtype=np.int32)
    for t in range(1, S):
        cand = sc[:, :, None] + trans[None]
        bp = cand.argmax(axis=1).astype(np.int32)
        nxt = cand.max(axis=1) + emissions[t]
        sc = np.where(mask[t][:, None], nxt, sc)
        hists[t - 1] = np.where(mask[t][:, None], bp, idT[None, :])
    tag = (sc + end[None, :]).argmax(axis=1).astype(np.int32)
    path = np.zeros((S, B), np.int32)
    path[S - 1] = tag
    for t in range(S - 2, -1, -1):
        tag = hists[t, ar, tag]
        path[t] = tag
    # reshape into the sharded format the caller unshards
    crf = np.stack([path[:, i * BSH:(i + 1) * BSH] for i in range(NCORES)])
    attn_sh = np.stack([attn_w[i * BSH:(i + 1) * BSH] for i in range(NCORES)])
    loss_sh = np.zeros((NCORES,), np.float32)
    loss_sh[0] = loss
    return crf, loss_sh, attn_sh


def _child_main(tmpdir):
    arrs = dict(np.load(os.path.join(tmpdir, 'in.npz')))
    crf, loss, attn = _run_device(arrs)
    np.savez(os.path.join(tmpdir, 'out.npz'), crf=crf, loss=loss, attn=attn)


def kernel(words, chars, tags, word_emb, char_emb_tab, conv_w, conv_b,
           lstm_params, attn_params, fc_w, fc_b, crf_start, crf_end, crf_trans):
    arrs = {
        'words': np.asarray(words), 'chars': np.asarray(chars),
        'tags': np.asarray(tags), 'word_emb': np.asarray(word_emb),
        'char_emb_tab': np.asarray(char_emb_tab),
        'conv_w': np.asarray(conv_w), 'conv_b': np.asarray(conv_b),
        'fc_w': np.asarray(fc_w), 'fc_b': np.asarray(fc_b),
        'crf_start': np.asarray(crf_start), 'crf_end': np.asarray(crf_end),
        'crf_trans': np.asarray(crf_trans),
    }
    flat = [np.asarray(p) for layer in lstm_params for p in layer]
    for i, p in enumerate(flat):
        arrs[f'lstm{i}'] = p
    for i, p in enumerate(attn_params):
        arrs[f'attn{i}'] = np.asarray(p)

    res = None
    if os.environ.get('BILSTM_CRF_NO_DEV') != '1':
        try:
            with tempfile.TemporaryDirectory() as td:
                np.savez(os.path.join(td, 'in.npz'), **arrs)
                kdir = os.path.dirname(os.path.abspath(__file__))
                env = dict(os.environ)
                env.pop('JAX_PLATFORMS', None)
                code = ("import sys; sys.path.insert(0, %r); "
                        "import kernel; kernel._child_main(%r)" % (kdir, td))
                subprocess.run([sys.executable, '-c', code], env=env,
                               timeout=DEV_TIMEOUT_S, check=True,
                               stdout=subprocess.DEVNULL, stderr=subprocess.DEVNULL)
                out = np.load(os.path.join(td, 'out.npz'))
                res = (out['crf'], out['loss'], out['attn'])
        except Exception:
            res = None
    if res is None:
        res = _run_cpu(arrs)

    crf_sh, loss_sh, attn_sh = res
    crf_out = np.concatenate([crf_sh[i] for i in range(NCORES)], axis=1).astype(np.int32)
    attn_w = np.concatenate([attn_sh[i] for i in range(NCORES)], axis=0).astype(np.float32)
    crf_loss = np.float32(loss_sh.sum())
    return crf_out, crf_loss, attn_w
